# revision 1
# baseline (speedup 1.0000x reference)
"""Disentangled self-attention (DeBERTa-style) on 8 TRN2 NeuronCores.

Problem: B=4, L=256, D=512, H=8, R=64 rel-pos buckets, DK=64.
Sharding: core c handles batch b=c//2, query rows l0=128*(c%2) .. l0+128.
No cross-core communication (output rows are disjoint).

Device dataflow per core (all matmuls accumulate f32 in PSUM):
  - projections in fp16, feature-major q/k ([o,l]) and token-major v ([m,o]),
    bias folded via an appended ones-row / bias-row (aug) on the host
  - scores psum A[l,h,m] = additive key mask (rank-1, seeds each bank)
      + q.k per head + transposed-accumulated content->position term (t1)
  - term2 psum B[l,m,h] = position->content one-hot matmuls (one per key m)
  - gathers use host-built one-hot matrices O1[r,l,m] / O2[r,m,l] (fp16)
  - softmax per head on DVE/ACT (exp with fused bias=-rowmax and fused sum)
  - p transposed per head via PE into pT[m,h,l] fp16
  - ctx via head-pair psums [128, 2*128]: v-seed matmul + one [128x128]@[128x2]
    matmul per (query row, head pair, key chunk) streaming rel_v fp16
  - output projection from the diagonal-extracted ctxT, f32 result to DRAM
"""

import sys

for _p in ("/opt/trn_rl_repo", "/root/.axon_site/_ro/trn_rl_repo"):
    if _p not in sys.path:
        sys.path.append(_p)

import numpy as np

import concourse.bass as bass
import concourse.tile as tile
from concourse import bacc, mybir
from concourse.bass_utils import run_bass_kernel_spmd
from concourse.masks import make_identity

B, L, D, H = 4, 256, 512, 8
R = 64
DK = D // H
LH = 128                      # query rows per core
NCORES = 8
SCALE = float(1.0 / (3.0 * np.sqrt(np.float32(DK))))
MASKVAL = -60000.0            # exp() underflows identically to the ref's -1e9

F16 = mybir.dt.float16
F32 = mybir.dt.float32
EXP = mybir.ActivationFunctionType.Exp
AX = mybir.AxisListType.X

O1_BLK = 16                   # query rows per streamed O1 block
O2_BLK = 16                   # key rows per streamed O2 block


def build_nc(phase=99, sub="all"):
    nc = bacc.Bacc(None, target_bir_lowering=False)

    # ---- DRAM I/O (per-core shard shapes) ----
    d_qT = nc.dram_tensor("qT", [513, LH], F16, kind="ExternalInput")
    d_kT = nc.dram_tensor("kT", [513, L], F16, kind="ExternalInput")
    d_vT = nc.dram_tensor("vT", [513, L], F16, kind="ExternalInput")
    d_WqT = nc.dram_tensor("WqT", [513, D], F16, kind="ExternalInput")
    d_WkT = nc.dram_tensor("WkT", [513, D], F16, kind="ExternalInput")
    d_WvT = nc.dram_tensor("WvT", [513, D], F16, kind="ExternalInput")
    d_WoT = nc.dram_tensor("WoT", [D, D], F16, kind="ExternalInput")
    d_rkT = nc.dram_tensor("relkT", [DK, H, R], F16, kind="ExternalInput")
    d_rqT = nc.dram_tensor("relqT", [DK, H, R], F16, kind="ExternalInput")
    # stacked-pair one-hots: row r + 64*j holds pair-member j (l=2p+j / m=2p+j)
    d_O1 = nc.dram_tensor("O1", [128, LH // 2, L], F16, kind="ExternalInput")
    d_O2 = nc.dram_tensor("O2", [128, L // 2, LH], F16, kind="ExternalInput")
    d_mask = nc.dram_tensor("maskfull", [1, H * L], F16, kind="ExternalInput")
    d_ones = nc.dram_tensor("ones16", [1, LH], F16, kind="ExternalInput")
    d_rv = nc.dram_tensor("rv", [LH, L, D], F16, kind="ExternalInput")
    d_out = nc.dram_tensor("out", [LH, D], F32, kind="ExternalOutput")

    with tile.TileContext(nc) as tc:
        with (
            tc.tile_pool(name="consts", bufs=1) as consts,
            tc.tile_pool(name="work", bufs=1) as work,
            tc.tile_pool(name="sm", bufs=3) as smp,
            tc.tile_pool(name="rvp", bufs=24) as rvp,
        ):
            dbg_ap = None

            # ---------- constants into SBUF ----------
            def load(name, dram, shape, dtype=F16):
                t = consts.tile(shape, dtype, tag=name, name=name)
                nc.gpsimd.dma_start(out=t[:], in_=dram)
                return t

            wq = [load(f"wq{i}", d_WqT[i * 128:(i + 1) * 128, :], [128, D]) for i in range(4)]
            wqb = load("wqb", d_WqT[512:513, :], [1, D])
            wk = [load(f"wk{i}", d_WkT[i * 128:(i + 1) * 128, :], [128, D]) for i in range(4)]
            wkb = load("wkb", d_WkT[512:513, :], [1, D])
            wv = [load(f"wv{i}", d_WvT[i * 128:(i + 1) * 128, :], [128, D]) for i in range(4)]
            wvb = load("wvb", d_WvT[512:513, :], [1, D])
            wo = [load(f"wo{i}", d_WoT[i * 128:(i + 1) * 128, :], [128, D]) for i in range(4)]
            xq = [load(f"xq{i}", d_qT[i * 128:(i + 1) * 128, :], [128, LH]) for i in range(4)]
            xqb = load("xqb", d_qT[512:513, :], [1, LH])
            xk = [load(f"xk{i}", d_kT[i * 128:(i + 1) * 128, :], [128, L]) for i in range(4)]
            xkb = load("xkb", d_kT[512:513, :], [1, L])
            xv = [load(f"xv{i}", d_vT[i * 128:(i + 1) * 128, :], [128, L]) for i in range(4)]
            xvb = load("xvb", d_vT[512:513, :], [1, L])
            rkT = load("rkT", d_rkT[:, :, :], [DK, H, R])
            rqT = load("rqT", d_rqT[:, :, :], [DK, H, R])
            maskf = load("maskf", d_mask[:, :], [1, H * L])
            ones16 = load("ones16", d_ones[:, :], [1, LH])

            o1t = consts.tile([128, LH // 2, L], F16, tag="o1t")
            nc.gpsimd.dma_start(out=o1t[:], in_=d_O1[:, :, :])
            o2t = consts.tile([128, L // 2, LH], F16, tag="o2t")
            nc.gpsimd.dma_start(out=o2t[:], in_=d_O2[:, :, :])

            id16 = consts.tile([128, 128], F16, tag="id16")
            make_identity(nc, id16[:])
            id32 = consts.tile([128, 128], F32, tag="id32")
            make_identity(nc, id32[:])

            if phase == 0:
                dbg_ap = xq[0][:]

            # ---------- projections ----------
            if phase >= 1:
                qf2 = work.tile([DK, H, LH], F16, tag="qf2", name="qf2")
                kf2 = work.tile([DK, H, L], F16, tag="kf2", name="kf2")
                vp = [work.tile([128, D], F16, tag=f"vp{i}", name=f"vp{i}") for i in range(2)]

                with tc.tile_pool(name="pproj", bufs=2, space="PSUM") as pproj:
                    for h in range(H):
                        hs = slice(h * DK, (h + 1) * DK)
                        ps = pproj.tile([DK, LH], F32, tag="pp", name="pp")
                        for kc in range(4):
                            nc.tensor.matmul(ps[:], wq[kc][:, hs],
                                             xq[kc][:], start=(kc == 0), stop=False)
                        nc.tensor.matmul(ps[:], wqb[:, hs],
                                         xqb[:], start=False, stop=True)
                        nc.vector.tensor_copy(qf2[:, h, :], ps[:])
                    for h in range(H):
                        hs = slice(h * DK, (h + 1) * DK)
                        ps = pproj.tile([DK, L], F32, tag="pp", name="pp")
                        for kc in range(4):
                            nc.tensor.matmul(ps[:], wk[kc][:, hs],
                                             xk[kc][:], start=(kc == 0), stop=False)
                        nc.tensor.matmul(ps[:], wkb[:, hs],
                                         xkb[:], start=False, stop=True)
                        nc.vector.tensor_copy(kf2[:, h, :], ps[:])
                    for mc in range(2):
                        ps = pproj.tile([128, D], F32, tag="pp", name="pp")
                        for kc in range(4):
                            nc.tensor.matmul(ps[:], xv[kc][:, mc * 128:(mc + 1) * 128],
                                             wv[kc][:], start=(kc == 0), stop=False)
                        nc.tensor.matmul(ps[:], xvb[:, mc * 128:(mc + 1) * 128],
                                         wvb[:], start=False, stop=True)
                        nc.vector.tensor_copy(vp[mc][:], ps[:])

                    # c2p/p2c with pair-member-major (j-major) column order so
                    # the block-diag rhs assembles from contiguous runs
                    c2pJ = work.tile([R, 2, H, LH // 2], F16, tag="c2pJ", name="c2pJ")
                    p2cJ = work.tile([R, 2, H, L // 2], F16, tag="p2cJ", name="p2cJ")
                    for h in range(H):
                        ps = pproj.tile([R, LH], F32, tag="pc", name="pc")
                        nc.tensor.matmul(ps[:], rkT[:, h, :],
                                         qf2[:, h, :].rearrange("d (p j) -> d j p", j=2),
                                         start=True, stop=True)
                        nc.vector.tensor_copy(c2pJ[:, 0, h, :], ps[:, 0:LH // 2])
                        nc.vector.tensor_copy(c2pJ[:, 1, h, :], ps[:, LH // 2:])
                        ps2 = pproj.tile([R, L], F32, tag="pc", name="pc")
                        nc.tensor.matmul(ps2[:], rqT[:, h, :],
                                         kf2[:, h, :].rearrange("d (p j) -> d j p", j=2),
                                         start=True, stop=True)
                        nc.vector.tensor_copy(p2cJ[:, 0, h, :], ps2[:, 0:L // 2])
                        nc.vector.tensor_copy(p2cJ[:, 1, h, :], ps2[:, L // 2:])

                # block-diagonal pair rhs: rows 0-63 even member (cols 0-7),
                # rows 64-127 odd member (cols 8-15); off-blocks are zero
                c2p2 = work.tile([128, 16, LH // 2], F16, tag="c2p2", name="c2p2")
                p2c2 = work.tile([128, 16, L // 2], F16, tag="p2c2", name="p2c2")
                nc.vector.memset(c2p2[:], 0.0)
                nc.vector.memset(p2c2[:], 0.0)
                # even members: same partitions, plain DVE copy; odd members:
                # partition shift via sb->sb DMA (contiguous 1-2KB per partition)
                nc.vector.tensor_copy(c2p2[0:64, 0:8, :], c2pJ[:, 0, :, :])
                nc.gpsimd.dma_start(out=c2p2[64:128, 8:16, :], in_=c2pJ[:, 1, :, :])
                nc.vector.tensor_copy(p2c2[0:64, 0:8, :], p2cJ[:, 0, :, :])
                nc.gpsimd.dma_start(out=p2c2[64:128, 8:16, :], in_=p2cJ[:, 1, :, :])

                if phase == 1:
                    dbg_ap = vp[0][:]

            # ---------- scores + softmax ----------
            _lv = {"qk": 0, "t1": 1, "tr": 2, "B": 3, "sm": 4, "all": 9}[sub]
            if phase >= 2:
                with tc.tile_pool(name="pscore", bufs=1, space="PSUM") as pscore:
                    A = pscore.tile([128, H, L], F32, tag="A", name="A")    # 4 banks
                    # mask seeds each bank (start=True covers 2 heads)
                    for h2 in range(0, H, 2):
                        nc.tensor.matmul(A[:, h2:h2 + 2, :], ones16[:],
                                         maskf[:, h2 * L:(h2 + 2) * L],
                                         start=True, stop=False)
                    for h in range(H):
                        nc.tensor.matmul(A[:, h, :], qf2[:, h, :], kf2[:, h, :],
                                         start=False, stop=False)

                    # t1: psum t1T[m, l, h] per m-chunk -> sb -> PE-transpose into A
                    t1s = [work.tile([128, LH, H], F32, tag=f"t1s{mc}", name=f"t1s{mc}")
                           for mc in range(2)]
                    for mc in range(2 if _lv >= 1 else 0):
                        t1 = pscore.tile([128, LH, H], F32, tag="big", name="big")
                        for p in range(LH // 2):
                            nc.tensor.matmul(t1[:, 2 * p:2 * p + 2, :],
                                             o1t[:, p, mc * 128:(mc + 1) * 128],
                                             c2p2[:, :, p],
                                             start=(p % 32 == 0), stop=(p % 32 == 31))
                        nc.vector.tensor_copy(t1s[mc][:], t1[:])
                    for mc in range(2 if _lv >= 2 else 0):
                        for h in range(H):
                            nc.tensor.matmul(A[:, h, mc * 128:(mc + 1) * 128],
                                             t1s[mc][:, :, h], id32[:],
                                             is_transpose=True, start=False,
                                             stop=(mc == 1 and h % 2 == 1))

                    # term2: B[l, m, h], one matmul per key m
                    Bp = pscore.tile([128, L, H], F32, tag="big", name="big")
                    for p in range(L // 2 if _lv >= 3 else 0):
                        nc.tensor.matmul(Bp[:, 2 * p:2 * p + 2, :], o2t[:, p, :],
                                         p2c2[:, :, p],
                                         start=(p % 32 == 0), stop=(p % 32 == 31))
                    B_sb = work.tile([128, L, H], F32, tag="B_sb", name="B_sb")
                    if _lv >= 3:
                        nc.vector.tensor_copy(B_sb[:], Bp[:])

                    # softmax per head
                    p16 = work.tile([128, H, L], F16, tag="p16", name="p16")
                    sums = work.tile([128, H], F32, tag="sums", name="sums")
                    recs = work.tile([128, H], F32, tag="recs", name="recs")
                    for h in range(H if _lv >= 4 else 0):
                        s = smp.tile([128, L], F32, tag="s", name="s")
                        nc.vector.tensor_add(s[:], A[:, h, :], B_sb[:, :, h])
                        nmax = smp.tile([128, 1], F32, tag="nmax", name="nmax")
                        nc.vector.reduce_max(nmax[:], s[:], axis=AX, negate=True)
                        e = smp.tile([128, L], F32, tag="e", name="e")
                        nc.scalar.activation(e[:], s[:], EXP, bias=nmax[:], scale=1.0,
                                             accum_out=sums[:, h:h + 1])
                        nc.vector.reciprocal(recs[:, h:h + 1], sums[:, h:h + 1])
                        nc.vector.tensor_scalar_mul(p16[:, h, :], e[:], recs[:, h:h + 1])

                if phase == 2:
                    dbg_ap = {0: A[:, 0, :], 1: t1s[0][:, :, 0], 2: A[:, 0, :],
                              3: B_sb[:, :, 0], 4: p16[:, 0, :], 9: B_sb[:, :, 0]}[_lv]
                if phase == 3:
                    dbg_ap = p16[:, 0, :]

            # ---------- ctx + output projection ----------
            if phase >= 4:
                with (
                    tc.tile_pool(name="pctx", bufs=1, space="PSUM") as pctx,
                    tc.tile_pool(name="ppt", bufs=2, space="PSUM") as ppt,
                ):
                    pT = [work.tile([128, H, LH], F16, tag=f"pT{c}", name=f"pT{c}")
                          for c in range(2)]
                    for c in range(2):
                        for h in range(H):
                            pps = ppt.tile([128, 128], F16, tag="pt", name="pt")
                            nc.tensor.matmul(pps[:], p16[:, h, c * 128:(c + 1) * 128],
                                             id16[:], is_transpose=True)
                            nc.vector.tensor_copy(pT[c][:, h, :], pps[:])

                    cp = [pctx.tile([128, 2 * LH], F32, tag=f"cp{hp}", name=f"cp{hp}")
                          for hp in range(4)]
                    nrv = LH if phase >= 5 else 0
                    for hp in range(4):
                        for c in range(2):
                            rhs = pT[c][:, 2 * hp:2 * hp + 2, :].rearrange("p hh l -> p l hh")
                            nc.tensor.matmul(cp[hp][:], vp[c][:, hp * 128:(hp + 1) * 128],
                                             rhs, start=(c == 0),
                                             stop=(c == 1 and nrv == 0))
                    for l in range(nrv):
                        rvt = rvp.tile([128, 2, D], F16, tag="rv", name="rv")
                        nc.sync.dma_start(out=rvt[:],
                                          in_=d_rv[l].rearrange("(c p) f -> p c f", p=128))
                        for hp in range(4):
                            for c in range(2):
                                nc.tensor.matmul(
                                    cp[hp][:, 2 * l:2 * l + 2],
                                    rvt[:, c, hp * 128:(hp + 1) * 128],
                                    pT[c][:, 2 * hp:2 * hp + 2, l:l + 1],
                                    start=False, stop=(c == 1 and l == LH - 1))

                    ctxT = [work.tile([128, LH], F16, tag=f"ctxT{hp}", name=f"ctxT{hp}")
                            for hp in range(4)]
                    for hp in range(4):
                        nc.vector.tensor_copy(
                            ctxT[hp][0:64, :],
                            cp[hp][0:64, :].rearrange("p (l hh) -> p hh l", hh=2)[:, 0, :])
                        nc.vector.tensor_copy(
                            ctxT[hp][64:128, :],
                            cp[hp][64:128, :].rearrange("p (l hh) -> p hh l", hh=2)[:, 1, :])
                    ops = pctx.tile([128, D], F32, tag="ops", name="ops")
                    for hp in range(4):
                        nc.tensor.matmul(ops[:], ctxT[hp][:], wo[hp][:],
                                         start=(hp == 0), stop=(hp == 3))
                    out_sb = work.tile([128, D], F32, tag="out_sb", name="out_sb")
                    nc.vector.tensor_copy(out_sb[:], ops[:])
                    nc.sync.dma_start(out=d_out[:, :], in_=out_sb[:])

            if phase < 4:
                dbg = work.tile([128, D], F32, tag="dbg", name="dbg")
                nc.vector.memset(dbg[:], 0.0)
                n = min(int(np.prod(dbg_ap.shape[1:])), D)
                nc.vector.tensor_copy(dbg[:dbg_ap.shape[0], 0:n], dbg_ap[:, 0:n])
                nc.sync.dma_start(out=d_out[:, :], in_=dbg[:])

    nc.finalize()
    return nc


_NC_CACHE = None


def _get_nc():
    global _NC_CACHE
    if _NC_CACHE is None:
        import os
        _NC_CACHE = build_nc(int(os.environ.get("KPHASE", "99")),
                             os.environ.get("KSUB", "all"))
    return _NC_CACHE


def host_prep(inputs, c):
    f16, f32 = np.float16, np.float32
    b, lh = c // 2, c % 2
    l0 = lh * LH
    q = np.asarray(inputs["query"][b], f32)
    k = np.asarray(inputs["key"][b], f32)
    v = np.asarray(inputs["value"][b], f32)
    mask = np.asarray(inputs["mask"][b])
    rp = np.asarray(inputs["rel_pos"][b], np.int64)
    rv = np.asarray(inputs["rel_v"][b], f32)

    d = {}
    d["qT"] = np.vstack([(q[l0:l0 + LH].T) * SCALE, np.ones((1, LH))]).astype(f16)
    d["kT"] = np.vstack([k.T, np.ones((1, L))]).astype(f16)
    d["vT"] = np.vstack([v.T, np.ones((1, L))]).astype(f16)
    d["WqT"] = np.vstack([np.asarray(inputs["Wq"], f32).T,
                          np.asarray(inputs["bq"], f32)[None, :] * SCALE]).astype(f16)
    d["WkT"] = np.vstack([np.asarray(inputs["Wk"], f32).T,
                          np.asarray(inputs["bk"], f32)[None, :]]).astype(f16)
    d["WvT"] = np.vstack([np.asarray(inputs["Wv"], f32).T,
                          np.asarray(inputs["bv"], f32)[None, :]]).astype(f16)
    d["WoT"] = np.asarray(inputs["Wo"], f32).T.astype(f16)
    d["relkT"] = np.ascontiguousarray(
        np.asarray(inputs["rel_k"], f32).transpose(2, 0, 1)).astype(f16)
    d["relqT"] = np.ascontiguousarray(
        np.asarray(inputs["rel_q"], f32).transpose(2, 0, 1) * SCALE).astype(f16)
    rp_c = rp[l0:l0 + LH]
    eye = np.eye(R, dtype=f16)
    O1 = eye[:, rp_c]                                   # [R, LH, L]
    O2 = eye[:, rp_c.T]                                 # [R, L, LH]
    d["O1"] = np.ascontiguousarray(
        O1.reshape(R, LH // 2, 2, L).transpose(2, 0, 1, 3).reshape(128, LH // 2, L))
    d["O2"] = np.ascontiguousarray(
        O2.reshape(R, L // 2, 2, LH).transpose(2, 0, 1, 3).reshape(128, L // 2, LH))
    mrow = np.where(mask, np.float16(MASKVAL), np.float16(0.0)).astype(f16)
    d["maskfull"] = np.tile(mrow, (1, H)).reshape(1, H * L)
    d["ones16"] = np.ones((1, LH), f16)
    d["rv"] = rv[l0:l0 + LH].astype(f16)                # [LH, L, D]
    return d


def kernel(**inputs) -> np.ndarray:
    nc = _get_nc()
    in_maps = [host_prep(inputs, c) for c in range(NCORES)]
    res = run_bass_kernel_spmd(nc, in_maps, core_ids=list(range(NCORES)))
    out = np.zeros((B, L, D), np.float32)
    for c in range(NCORES):
        b, lh = c // 2, c % 2
        out[b, lh * LH:(lh + 1) * LH] = res.results[c]["out"]
    out += np.asarray(inputs["bo"], np.float32)[None, None, :]
    return out



# revision 2
# speedup vs baseline: 1.3686x; 1.3686x over previous
"""Disentangled self-attention (DeBERTa-style) on 8 TRN2 NeuronCores.

Problem: B=4, L=256, D=512, H=8, R=64 rel-pos buckets, DK=64.
Sharding: core c handles batch b=c//2, query rows l0=128*(c%2) .. l0+128.
No cross-core communication (output rows are disjoint).

Device dataflow per core (all matmuls accumulate f32 in PSUM):
  - projections in fp16, feature-major q/k ([o,l]) and token-major v ([m,o]),
    bias folded via an appended ones-row / bias-row (aug) on the host
  - scores psum A[l,h,m] = additive key mask (rank-1, seeds each bank)
      + q.k per head + transposed-accumulated content->position term (t1)
  - term2 psum B[l,m,h] = position->content one-hot matmuls (one per key m)
  - gathers use host-built one-hot matrices O1[r,l,m] / O2[r,m,l] (fp8e3m4,
    exact for 0/1) as FWL weights against fp16 c2p/p2c moving operands
  - softmax per head on DVE/ACT (exp with fused bias=-rowmax and fused sum)
  - p transposed per head via PE into pT[m,h,l] fp16
  - ctx via head-pair psums [128, 2*128]: v-seed matmul + one fp8-weight
    [128x128]@[128x2] matmul per (query row, head pair, key chunk); rel_v
    streams as fp8e3m4 weights (halves HBM traffic, 4x FWL), p stays fp16
  - output projection from the diagonal-extracted ctxT, f32 result to DRAM
"""

import sys

for _p in ("/opt/trn_rl_repo", "/root/.axon_site/_ro/trn_rl_repo"):
    if _p not in sys.path:
        sys.path.append(_p)

import numpy as np
import ml_dtypes

import concourse.bass as bass
import concourse.tile as tile
from concourse import bacc, mybir
from concourse.bass_utils import run_bass_kernel_spmd
from concourse.masks import make_identity

B, L, D, H = 4, 256, 512, 8
R = 64
DK = D // H
LH = 128                      # query rows per core
NCORES = 8
SCALE = float(1.0 / (3.0 * np.sqrt(np.float32(DK))))
MASKVAL = -60000.0            # exp() underflows identically to the ref's -1e9
RVG = 8                       # query rows per rel_v DMA group
NG = LH // RVG

F16 = mybir.dt.float16
F32 = mybir.dt.float32
E3 = mybir.dt.float8e3
EXP = mybir.ActivationFunctionType.Exp
AX = mybir.AxisListType.X

NP_E3 = ml_dtypes.float8_e3m4


def build_nc(phase=99, sub="all"):
    nc = bacc.Bacc(None, target_bir_lowering=False)

    # ---- DRAM I/O (per-core shard shapes) ----
    d_qT = nc.dram_tensor("qT", [513, LH], F16, kind="ExternalInput")
    d_kT = nc.dram_tensor("kT", [513, L], F16, kind="ExternalInput")
    d_vT = nc.dram_tensor("vT", [513, L], F16, kind="ExternalInput")
    d_WqT = nc.dram_tensor("WqT", [513, D], F16, kind="ExternalInput")
    d_WkT = nc.dram_tensor("WkT", [513, D], F16, kind="ExternalInput")
    d_WvT = nc.dram_tensor("WvT", [513, D], F16, kind="ExternalInput")
    d_WoT = nc.dram_tensor("WoT", [D, D], F16, kind="ExternalInput")
    d_rkT = nc.dram_tensor("relkT", [DK, H, R], F16, kind="ExternalInput")
    d_rqT = nc.dram_tensor("relqT", [DK, H, R], F16, kind="ExternalInput")
    # stacked-pair one-hots: row r + 64*j holds pair-member j (l=2p+j / m=2p+j)
    d_O1 = nc.dram_tensor("O1", [128, LH // 2, L], E3, kind="ExternalInput")
    d_O2 = nc.dram_tensor("O2", [128, L // 2, LH], E3, kind="ExternalInput")
    d_mask = nc.dram_tensor("maskfull", [1, H * L], F16, kind="ExternalInput")
    d_ones = nc.dram_tensor("ones16", [1, LH], F16, kind="ExternalInput")
    # rel_v fp8, grouped RVG rows per DMA: [g, m0, j, c, f]
    d_rv = nc.dram_tensor("rv", [NG, 128, RVG, 2, D], E3, kind="ExternalInput")
    d_out = nc.dram_tensor("out", [LH, D], F32, kind="ExternalOutput")

    with tile.TileContext(nc) as tc:
        with (
            tc.tile_pool(name="consts", bufs=1) as consts,
            tc.tile_pool(name="work", bufs=1) as work,
            tc.tile_pool(name="sm", bufs=3) as smp,
            tc.tile_pool(name="rvp", bufs=4) as rvp,
        ):
            dbg_ap = None

            # ---------- constants into SBUF ----------
            def load(name, dram, shape, dtype=F16, eng=None):
                t = consts.tile(shape, dtype, tag=name, name=name)
                (eng or nc.scalar).dma_start(out=t[:], in_=dram)
                return t

            def c4(d):  # [512, N] dram -> [128, 4, N] sbuf AP
                return d[0:512].rearrange("(c p) f -> p c f", p=128)

            wqt = load("wqt", c4(d_WqT), [128, 4, D])
            wqb = load("wqb", d_WqT[512:513, :], [1, D])
            wkt = load("wkt", c4(d_WkT), [128, 4, D])
            wkb = load("wkb", d_WkT[512:513, :], [1, D])
            wvt = load("wvt", c4(d_WvT), [128, 4, D])
            wvb = load("wvb", d_WvT[512:513, :], [1, D])
            wot = load("wot", c4(d_WoT), [128, 4, D])
            xqt = load("xqt", c4(d_qT), [128, 4, LH])
            xqb = load("xqb", d_qT[512:513, :], [1, LH])
            xkt = load("xkt", c4(d_kT), [128, 4, L])
            xkb = load("xkb", d_kT[512:513, :], [1, L])
            xvt = load("xvt", c4(d_vT), [128, 4, L])
            xvb = load("xvb", d_vT[512:513, :], [1, L])
            rkT = load("rkT", d_rkT[:, :, :], [DK, H, R])
            rqT = load("rqT", d_rqT[:, :, :], [DK, H, R])
            maskf = load("maskf", d_mask[:, :], [1, H * L])
            ones16 = load("ones16", d_ones[:, :], [1, LH])

            o1t = consts.tile([128, LH // 2, L], E3, tag="o1t")
            nc.gpsimd.dma_start(out=o1t[:], in_=d_O1[:, :, :])
            o2t = consts.tile([128, L // 2, LH], E3, tag="o2t")
            nc.gpsimd.dma_start(out=o2t[:], in_=d_O2[:, :, :])

            id16 = consts.tile([128, 128], F16, tag="id16")
            make_identity(nc, id16[:])
            id32 = consts.tile([128, 128], F32, tag="id32")
            make_identity(nc, id32[:])

            if phase == 0:
                dbg_ap = xqt[:, 0, :]

            # ---------- projections ----------
            if phase >= 1:
                qf2 = work.tile([DK, H, LH], F16, tag="qf2", name="qf2")
                kf2 = work.tile([DK, H, L], F16, tag="kf2", name="kf2")
                vp = [work.tile([128, D], F16, tag=f"vp{i}", name=f"vp{i}") for i in range(2)]

                with tc.tile_pool(name="pproj", bufs=2, space="PSUM") as pproj:
                    for h in range(H):
                        hs = slice(h * DK, (h + 1) * DK)
                        ps = pproj.tile([DK, LH], F32, tag="pp", name="pp")
                        for kc in range(4):
                            nc.tensor.matmul(ps[:], wqt[:, kc, hs],
                                             xqt[:, kc, :], start=(kc == 0), stop=False)
                        nc.tensor.matmul(ps[:], wqb[:, hs],
                                         xqb[:], start=False, stop=True)
                        nc.vector.tensor_copy(qf2[:, h, :], ps[:])
                    for h in range(H):
                        hs = slice(h * DK, (h + 1) * DK)
                        ps = pproj.tile([DK, L], F32, tag="pp", name="pp")
                        for kc in range(4):
                            nc.tensor.matmul(ps[:], wkt[:, kc, hs],
                                             xkt[:, kc, :], start=(kc == 0), stop=False)
                        nc.tensor.matmul(ps[:], wkb[:, hs],
                                         xkb[:], start=False, stop=True)
                        nc.vector.tensor_copy(kf2[:, h, :], ps[:])
                    for mc in range(2):
                        ps = pproj.tile([128, D], F32, tag="pp", name="pp")
                        for kc in range(4):
                            nc.tensor.matmul(ps[:], xvt[:, kc, mc * 128:(mc + 1) * 128],
                                             wvt[:, kc, :], start=(kc == 0), stop=False)
                        nc.tensor.matmul(ps[:], xvb[:, mc * 128:(mc + 1) * 128],
                                         wvb[:], start=False, stop=True)
                        nc.vector.tensor_copy(vp[mc][:], ps[:])

                    # c2p/p2c with pair-member-major (j-major) column order so
                    # the block-diag rhs assembles from contiguous runs
                    c2pJ = work.tile([R, 2, H, LH // 2], F16, tag="c2pJ", name="c2pJ")
                    p2cJ = work.tile([R, 2, H, L // 2], F16, tag="p2cJ", name="p2cJ")
                    for h in range(H):
                        ps = pproj.tile([R, LH], F32, tag="pc", name="pc")
                        nc.tensor.matmul(ps[:], rkT[:, h, :],
                                         qf2[:, h, :].rearrange("d (p j) -> d j p", j=2),
                                         start=True, stop=True)
                        nc.vector.tensor_copy(c2pJ[:, 0, h, :], ps[:, 0:LH // 2])
                        nc.vector.tensor_copy(c2pJ[:, 1, h, :], ps[:, LH // 2:])
                        ps2 = pproj.tile([R, L], F32, tag="pc", name="pc")
                        nc.tensor.matmul(ps2[:], rqT[:, h, :],
                                         kf2[:, h, :].rearrange("d (p j) -> d j p", j=2),
                                         start=True, stop=True)
                        nc.vector.tensor_copy(p2cJ[:, 0, h, :], ps2[:, 0:L // 2])
                        nc.vector.tensor_copy(p2cJ[:, 1, h, :], ps2[:, L // 2:])

                # block-diagonal pair rhs: rows 0-63 even member (cols 0-7),
                # rows 64-127 odd member (cols 8-15); off-blocks are zero
                c2p2 = work.tile([128, 16, LH // 2], F16, tag="c2p2", name="c2p2")
                p2c2 = work.tile([128, 16, L // 2], F16, tag="p2c2", name="p2c2")
                nc.vector.memset(c2p2[:], 0.0)
                nc.vector.memset(p2c2[:], 0.0)
                # even members: same partitions, plain DVE copy; odd members:
                # partition shift via sb->sb DMA (contiguous 1-2KB per partition)
                nc.vector.tensor_copy(c2p2[0:64, 0:8, :], c2pJ[:, 0, :, :])
                nc.gpsimd.dma_start(out=c2p2[64:128, 8:16, :], in_=c2pJ[:, 1, :, :])
                nc.vector.tensor_copy(p2c2[0:64, 0:8, :], p2cJ[:, 0, :, :])
                nc.gpsimd.dma_start(out=p2c2[64:128, 8:16, :], in_=p2cJ[:, 1, :, :])

                if phase == 1:
                    dbg_ap = vp[0][:]

            # ---------- scores + softmax ----------
            _lv = {"qk": 0, "t1": 1, "tr": 2, "B": 3, "sm": 4, "all": 9}[sub]
            if phase >= 2:
                with tc.tile_pool(name="pscore", bufs=1, space="PSUM") as pscore:
                    A = pscore.tile([128, H, L], F32, tag="A", name="A")    # 4 banks
                    # mask seeds each bank (start=True covers 2 heads)
                    for h2 in range(0, H, 2):
                        nc.tensor.matmul(A[:, h2:h2 + 2, :], ones16[:],
                                         maskf[:, h2 * L:(h2 + 2) * L],
                                         start=True, stop=False)
                    for h in range(H):
                        nc.tensor.matmul(A[:, h, :], qf2[:, h, :], kf2[:, h, :],
                                         start=False, stop=False)

                    # t1: psum t1T[m, l, h] per m-chunk -> sb -> PE-transpose into A
                    t1s = [work.tile([128, LH, H], F32, tag=f"t1s{mc}", name=f"t1s{mc}")
                           for mc in range(2)]
                    for mc in range(2 if _lv >= 1 else 0):
                        t1 = pscore.tile([128, LH, H], F32, tag="big", name="big")
                        for p in range(LH // 2):
                            nc.tensor.matmul(t1[:, 2 * p:2 * p + 2, :],
                                             o1t[:, p, mc * 128:(mc + 1) * 128],
                                             c2p2[:, :, p],
                                             start=(p % 32 == 0), stop=(p % 32 == 31))
                        nc.vector.tensor_copy(t1s[mc][:], t1[:])
                    for mc in range(2 if _lv >= 2 else 0):
                        for h in range(H):
                            nc.tensor.matmul(A[:, h, mc * 128:(mc + 1) * 128],
                                             t1s[mc][:, :, h], id32[:],
                                             is_transpose=True, start=False,
                                             stop=(mc == 1 and h % 2 == 1))

                    # term2: B[l, m, h], one matmul per key m
                    Bp = pscore.tile([128, L, H], F32, tag="big", name="big")
                    for p in range(L // 2 if _lv >= 3 else 0):
                        nc.tensor.matmul(Bp[:, 2 * p:2 * p + 2, :], o2t[:, p, :],
                                         p2c2[:, :, p],
                                         start=(p % 32 == 0), stop=(p % 32 == 31))
                    B_sb = work.tile([128, L, H], F32, tag="B_sb", name="B_sb")
                    if _lv >= 3:
                        nc.vector.tensor_copy(B_sb[:], Bp[:])

                    # softmax per head
                    p16 = work.tile([128, H, L], F16, tag="p16", name="p16")
                    sums = work.tile([128, H], F32, tag="sums", name="sums")
                    recs = work.tile([128, H], F32, tag="recs", name="recs")
                    for h in range(H if _lv >= 4 else 0):
                        s = smp.tile([128, L], F32, tag="s", name="s")
                        nc.vector.tensor_add(s[:], A[:, h, :], B_sb[:, :, h])
                        nmax = smp.tile([128, 1], F32, tag="nmax", name="nmax")
                        nc.vector.reduce_max(nmax[:], s[:], axis=AX, negate=True)
                        e = smp.tile([128, L], F32, tag="e", name="e")
                        nc.scalar.activation(e[:], s[:], EXP, bias=nmax[:], scale=1.0,
                                             accum_out=sums[:, h:h + 1])
                        nc.vector.reciprocal(recs[:, h:h + 1], sums[:, h:h + 1])
                        nc.vector.tensor_scalar_mul(p16[:, h, :], e[:], recs[:, h:h + 1])

                if phase == 2:
                    dbg_ap = {0: A[:, 0, :], 1: t1s[0][:, :, 0], 2: A[:, 0, :],
                              3: B_sb[:, :, 0], 4: p16[:, 0, :], 9: B_sb[:, :, 0]}[_lv]
                if phase == 3:
                    dbg_ap = p16[:, 0, :]

            # ---------- ctx + output projection ----------
            if phase >= 4:
                with (
                    tc.tile_pool(name="pctx", bufs=1, space="PSUM") as pctx,
                    tc.tile_pool(name="ppt", bufs=2, space="PSUM") as ppt,
                ):
                    pT = [work.tile([128, H, LH], F16, tag=f"pT{c}", name=f"pT{c}")
                          for c in range(2)]
                    for c in range(2):
                        for h in range(H):
                            pps = ppt.tile([128, 128], F16, tag="pt", name="pt")
                            nc.tensor.matmul(pps[:], p16[:, h, c * 128:(c + 1) * 128],
                                             id16[:], is_transpose=True)
                            nc.vector.tensor_copy(pT[c][:, h, :], pps[:])

                    cp = [pctx.tile([128, 2 * LH], F32, tag=f"cp{hp}", name=f"cp{hp}")
                          for hp in range(4)]
                    nrv = NG if phase >= 5 else 0
                    for hp in range(4):
                        for c in range(2):
                            rhs = pT[c][:, 2 * hp:2 * hp + 2, :].rearrange("p hh l -> p l hh")
                            nc.tensor.matmul(cp[hp][:], vp[c][:, hp * 128:(hp + 1) * 128],
                                             rhs, start=(c == 0),
                                             stop=(c == 1 and nrv == 0))
                    for g in range(nrv):
                        rvt = rvp.tile([128, RVG, 2, D], E3, tag="rv", name="rv")
                        nc.sync.dma_start(out=rvt[:], in_=d_rv[g])
                        for j in range(RVG):
                            l = RVG * g + j
                            for hp in range(4):
                                for c in range(2):
                                    nc.tensor.matmul(
                                        cp[hp][:, 2 * l:2 * l + 2],
                                        rvt[:, j, c, hp * 128:(hp + 1) * 128],
                                        pT[c][:, 2 * hp:2 * hp + 2, l:l + 1],
                                        start=False, stop=(c == 1 and l == LH - 1))

                    ctxT = [work.tile([128, LH], F16, tag=f"ctxT{hp}", name=f"ctxT{hp}")
                            for hp in range(4)]
                    for hp in range(4):
                        nc.vector.tensor_copy(
                            ctxT[hp][0:64, :],
                            cp[hp][0:64, :].rearrange("p (l hh) -> p hh l", hh=2)[:, 0, :])
                        nc.vector.tensor_copy(
                            ctxT[hp][64:128, :],
                            cp[hp][64:128, :].rearrange("p (l hh) -> p hh l", hh=2)[:, 1, :])
                    ops = pctx.tile([128, D], F32, tag="ops", name="ops")
                    for hp in range(4):
                        nc.tensor.matmul(ops[:], ctxT[hp][:], wot[:, hp, :],
                                         start=(hp == 0), stop=(hp == 3))
                    out_sb = work.tile([128, D], F32, tag="out_sb", name="out_sb")
                    nc.vector.tensor_copy(out_sb[:], ops[:])
                    nc.sync.dma_start(out=d_out[:, :], in_=out_sb[:])

            if phase < 4:
                dbg = work.tile([128, D], F32, tag="dbg", name="dbg")
                nc.vector.memset(dbg[:], 0.0)
                n = min(int(np.prod(dbg_ap.shape[1:])), D)
                nc.vector.tensor_copy(dbg[:dbg_ap.shape[0], 0:n], dbg_ap[:, 0:n])
                nc.sync.dma_start(out=d_out[:, :], in_=dbg[:])

    nc.finalize()
    return nc


_NC_CACHE = None


def _get_nc():
    global _NC_CACHE
    if _NC_CACHE is None:
        import os
        _NC_CACHE = build_nc(int(os.environ.get("KPHASE", "99")),
                             os.environ.get("KSUB", "all"))
    return _NC_CACHE


def host_prep(inputs, c):
    f16, f32 = np.float16, np.float32
    b, lh = c // 2, c % 2
    l0 = lh * LH
    q = np.asarray(inputs["query"][b], f32)
    k = np.asarray(inputs["key"][b], f32)
    v = np.asarray(inputs["value"][b], f32)
    mask = np.asarray(inputs["mask"][b])
    rp = np.asarray(inputs["rel_pos"][b], np.int64)
    rv = np.asarray(inputs["rel_v"][b], f32)

    d = {}
    d["qT"] = np.vstack([(q[l0:l0 + LH].T) * SCALE, np.ones((1, LH))]).astype(f16)
    d["kT"] = np.vstack([k.T, np.ones((1, L))]).astype(f16)
    d["vT"] = np.vstack([v.T, np.ones((1, L))]).astype(f16)
    d["WqT"] = np.vstack([np.asarray(inputs["Wq"], f32).T,
                          np.asarray(inputs["bq"], f32)[None, :] * SCALE]).astype(f16)
    d["WkT"] = np.vstack([np.asarray(inputs["Wk"], f32).T,
                          np.asarray(inputs["bk"], f32)[None, :]]).astype(f16)
    d["WvT"] = np.vstack([np.asarray(inputs["Wv"], f32).T,
                          np.asarray(inputs["bv"], f32)[None, :]]).astype(f16)
    d["WoT"] = np.asarray(inputs["Wo"], f32).T.astype(f16)
    d["relkT"] = np.ascontiguousarray(
        np.asarray(inputs["rel_k"], f32).transpose(2, 0, 1)).astype(f16)
    d["relqT"] = np.ascontiguousarray(
        np.asarray(inputs["rel_q"], f32).transpose(2, 0, 1) * SCALE).astype(f16)
    rp_c = rp[l0:l0 + LH]
    eye = np.eye(R, dtype=f32)
    O1 = eye[:, rp_c]                                   # [R, LH, L]
    O2 = eye[:, rp_c.T]                                 # [R, L, LH]
    d["O1"] = np.ascontiguousarray(
        O1.reshape(R, LH // 2, 2, L).transpose(2, 0, 1, 3)
        .reshape(128, LH // 2, L)).astype(NP_E3)
    d["O2"] = np.ascontiguousarray(
        O2.reshape(R, L // 2, 2, LH).transpose(2, 0, 1, 3)
        .reshape(128, L // 2, LH)).astype(NP_E3)
    mrow = np.where(mask, np.float16(MASKVAL), np.float16(0.0)).astype(f16)
    d["maskfull"] = np.tile(mrow, (1, H)).reshape(1, H * L)
    d["ones16"] = np.ones((1, LH), f16)
    # rel_v: [g, m0, j, c, f] where l = RVG*g + j, m = c*128 + m0
    rv_c = rv[l0:l0 + LH]                               # [LH, L, D]
    d["rv"] = np.ascontiguousarray(
        rv_c.reshape(NG, RVG, 2, 128, D).transpose(0, 3, 1, 2, 4)).astype(NP_E3)
    return d


def kernel(**inputs) -> np.ndarray:
    nc = _get_nc()
    in_maps = [host_prep(inputs, c) for c in range(NCORES)]
    res = run_bass_kernel_spmd(nc, in_maps, core_ids=list(range(NCORES)))
    out = np.zeros((B, L, D), np.float32)
    for c in range(NCORES):
        b, lh = c // 2, c % 2
        out[b, lh * LH:(lh + 1) * LH] = res.results[c]["out"]
    out += np.asarray(inputs["bo"], np.float32)[None, None, :]
    return out


# revision 4
# speedup vs baseline: 1.5814x; 1.1555x over previous
"""Disentangled self-attention (DeBERTa-style) on 8 TRN2 NeuronCores.

Problem: B=4, L=256, D=512, H=8, R=64 rel-pos buckets, DK=64.
Sharding: core c handles batch b=c//2, query rows l0=128*(c%2) .. l0+128.
No cross-core communication (output rows are disjoint).

Device dataflow per core (all matmuls accumulate f32 in PSUM):
  - constants arrive as 3 packed blobs (128-part / 64-part / 1-part) to
    minimize DMA-issue serialization at kernel start
  - projections in fp16, feature-major q/k ([o,l]) and token-major v ([m,o]),
    bias folded via an appended ones-row / bias-row (aug) on the host
  - scores psum A[l,h,m] = additive key mask (rank-1, seeds each bank)
      + q.k per head + transposed-accumulated content->position term (t1)
  - term2 psum B[l,m,h] = position->content one-hot matmuls (one per key m),
    issued before t1 so softmax can start as soon as each A bank closes
  - gathers use host-built one-hot matrices O1[r,l,m] / O2[r,m,l] (fp8e3m4,
    exact for 0/1) as weights against fp16 c2p/p2c moving operands
  - softmax per head-pair on DVE/ACT, pipelined under the PE transpose loop
  - p transposed per head via DMA-transpose XBAR (keeps PE free)
  - ctx via head-pair psums [128, 2*128]: v-seed matmul + one fp8-weight
    [128x128]@[128x2] matmul per (query row, head pair, key chunk); rel_v
    streams as fp8e3m4 weights (halves HBM traffic), p stays fp16
  - output projection from the diagonal-extracted ctxT, f32 result to DRAM
"""

import sys

for _p in ("/opt/trn_rl_repo", "/root/.axon_site/_ro/trn_rl_repo"):
    if _p not in sys.path:
        sys.path.append(_p)

import numpy as np
import ml_dtypes

import concourse.bass as bass
import concourse.tile as tile
from concourse import bacc, mybir
from concourse.bass_utils import run_bass_kernel_spmd
from concourse.masks import make_identity

B, L, D, H = 4, 256, 512, 8
R = 64
DK = D // H
LH = 128                      # query rows per core
NCORES = 8
SCALE = float(1.0 / (3.0 * np.sqrt(np.float32(DK))))
MASKVAL = -60000.0            # exp() underflows identically to the ref's -1e9
RVG = 8                       # query rows per rel_v DMA group
NG = LH // RVG

F16 = mybir.dt.float16
F32 = mybir.dt.float32
E3 = mybir.dt.float8e3
EXP = mybir.ActivationFunctionType.Exp
AX = mybir.AxisListType.X

NP_E3 = ml_dtypes.float8_e3m4

# blobA column offsets (128-partition consts, fp16)
A_WQ, A_WK, A_WV, A_WO = 0, 2048, 4096, 6144
A_XQ, A_XK, A_XV = 8192, 8704, 9728
A_END = 10752
# blobC column offsets (1-partition consts, fp16)
C_WQB, C_WKB, C_WVB = 0, 512, 1024
C_XQB, C_XKB, C_XVB = 1536, 1664, 1920
C_MASK, C_ONES = 2176, 4224
C_END = 4352


def build_nc(phase=99, sub="all"):
    nc = bacc.Bacc(None, target_bir_lowering=False)

    # ---- DRAM I/O (per-core shard shapes) ----
    d_bA = nc.dram_tensor("blobA", [128, A_END], F16, kind="ExternalInput")
    d_bB = nc.dram_tensor("blobB", [DK, 2 * 512], F16, kind="ExternalInput")
    d_bC = nc.dram_tensor("blobC", [1, C_END], F16, kind="ExternalInput")
    # stacked-pair one-hots: row r + 64*j holds pair-member j (l=2p+j / m=2p+j)
    d_O1 = nc.dram_tensor("O1", [128, LH // 2, L], E3, kind="ExternalInput")
    d_O2 = nc.dram_tensor("O2", [128, L // 2, LH], E3, kind="ExternalInput")
    # rel_v fp8, grouped RVG rows per DMA: [g, m0, j, c, f]
    d_rv = nc.dram_tensor("rv", [NG, 128, RVG, 2, D], E3, kind="ExternalInput")
    d_out = nc.dram_tensor("out", [LH, D], F32, kind="ExternalOutput")

    with tile.TileContext(nc) as tc:
        with (
            tc.tile_pool(name="consts", bufs=1) as consts,
            tc.tile_pool(name="work", bufs=1) as work,
            tc.tile_pool(name="sm", bufs=2) as smp,
            tc.tile_pool(name="rvp", bufs=6) as rvp,
        ):
            dbg_ap = None

            bA = consts.tile([128, A_END], F16, tag="bA", name="bA")
            nc.sync.dma_start(out=bA[:], in_=d_bA[:, :])
            bB = consts.tile([DK, 2 * 512], F16, tag="bB", name="bB")
            nc.sync.dma_start(out=bB[:], in_=d_bB[:, :])
            bC = consts.tile([1, C_END], F16, tag="bC", name="bC")
            nc.sync.dma_start(out=bC[:], in_=d_bC[:, :])

            o1t = consts.tile([128, LH // 2, L], E3, tag="o1t")
            nc.gpsimd.dma_start(out=o1t[:], in_=d_O1[:, :, :])
            o2t = consts.tile([128, L // 2, LH], E3, tag="o2t")
            nc.gpsimd.dma_start(out=o2t[:], in_=d_O2[:, :, :])

            id32 = consts.tile([128, 128], F32, tag="id32")
            make_identity(nc, id32[:])

            if phase == 0:
                dbg_ap = bA[:, A_XQ:A_XQ + 128]

            # ---------- projections ----------
            if phase >= 1:
                qf2 = work.tile([DK, H, LH], F16, tag="qf2", name="qf2")
                kf2 = work.tile([DK, H, L], F16, tag="kf2", name="kf2")
                vp = [work.tile([128, D], F16, tag=f"vp{i}", name=f"vp{i}") for i in range(2)]

                with tc.tile_pool(name="pproj", bufs=3, space="PSUM") as pproj:
                    for h in range(H):
                        o = A_WQ + h * 64
                        ps = pproj.tile([DK, LH], F32, tag="pp", name="pp")
                        for kc in range(4):
                            nc.tensor.matmul(ps[:], bA[:, o + kc * 512:o + kc * 512 + 64],
                                             bA[:, A_XQ + kc * 128:A_XQ + (kc + 1) * 128],
                                             start=(kc == 0), stop=False)
                        nc.tensor.matmul(ps[:], bC[:, C_WQB + h * 64:C_WQB + h * 64 + 64],
                                         bC[:, C_XQB:C_XQB + LH], start=False, stop=True)
                        nc.vector.tensor_copy(qf2[:, h, :], ps[:])
                    for h in range(H):
                        o = A_WK + h * 64
                        ps = pproj.tile([DK, L], F32, tag="pp", name="pp")
                        for kc in range(4):
                            nc.tensor.matmul(ps[:], bA[:, o + kc * 512:o + kc * 512 + 64],
                                             bA[:, A_XK + kc * 256:A_XK + (kc + 1) * 256],
                                             start=(kc == 0), stop=False)
                        nc.tensor.matmul(ps[:], bC[:, C_WKB + h * 64:C_WKB + h * 64 + 64],
                                         bC[:, C_XKB:C_XKB + L], start=False, stop=True)
                        nc.vector.tensor_copy(kf2[:, h, :], ps[:])
                    for mc in range(2):
                        ps = pproj.tile([128, D], F32, tag="pp", name="pp")
                        for kc in range(4):
                            nc.tensor.matmul(
                                ps[:],
                                bA[:, A_XV + kc * 256 + mc * 128:A_XV + kc * 256 + mc * 128 + 128],
                                bA[:, A_WV + kc * 512:A_WV + (kc + 1) * 512],
                                start=(kc == 0), stop=False)
                        nc.tensor.matmul(ps[:], bC[:, C_XVB + mc * 128:C_XVB + mc * 128 + 128],
                                         bC[:, C_WVB:C_WVB + 512], start=False, stop=True)
                        nc.vector.tensor_copy(vp[mc][:], ps[:])

                    # c2p/p2c with pair-member-major (j-major) column order so
                    # the block-diag rhs assembles from contiguous runs
                    c2pJ = work.tile([R, 2, H, LH // 2], F16, tag="c2pJ", name="c2pJ")
                    p2cJ = work.tile([R, 2, H, L // 2], F16, tag="p2cJ", name="p2cJ")
                    for h in range(H):
                        ps = pproj.tile([R, LH], F32, tag="pc", name="pc")
                        nc.tensor.matmul(ps[:], bB[:, h * 64:(h + 1) * 64],
                                         qf2[:, h, :].rearrange("d (p j) -> d j p", j=2),
                                         start=True, stop=True)
                        nc.vector.tensor_copy(c2pJ[:, 0, h, :], ps[:, 0:LH // 2])
                        nc.vector.tensor_copy(c2pJ[:, 1, h, :], ps[:, LH // 2:])
                        ps2 = pproj.tile([R, L], F32, tag="pc", name="pc")
                        nc.tensor.matmul(ps2[:], bB[:, 512 + h * 64:512 + (h + 1) * 64],
                                         kf2[:, h, :].rearrange("d (p j) -> d j p", j=2),
                                         start=True, stop=True)
                        nc.vector.tensor_copy(p2cJ[:, 0, h, :], ps2[:, 0:L // 2])
                        nc.vector.tensor_copy(p2cJ[:, 1, h, :], ps2[:, L // 2:])

                # block-diagonal pair rhs: rows 0-63 even member (cols 0-7),
                # rows 64-127 odd member (cols 8-15); off-blocks are zero
                c2p2 = work.tile([128, 16, LH // 2], F16, tag="c2p2", name="c2p2")
                p2c2 = work.tile([128, 16, L // 2], F16, tag="p2c2", name="p2c2")
                nc.vector.memset(c2p2[0:64, 8:16, :], 0.0)
                nc.vector.memset(c2p2[64:128, 0:8, :], 0.0)
                nc.vector.memset(p2c2[0:64, 8:16, :], 0.0)
                nc.vector.memset(p2c2[64:128, 0:8, :], 0.0)
                # even members: same partitions, plain DVE copy; odd members:
                # partition shift via sb->sb DMA (contiguous 0.5-1KB per partition)
                nc.vector.tensor_copy(c2p2[0:64, 0:8, :], c2pJ[:, 0, :, :])
                nc.gpsimd.dma_start(out=c2p2[64:128, 8:16, :], in_=c2pJ[:, 1, :, :])
                nc.vector.tensor_copy(p2c2[0:64, 0:8, :], p2cJ[:, 0, :, :])
                nc.gpsimd.dma_start(out=p2c2[64:128, 8:16, :], in_=p2cJ[:, 1, :, :])

                if phase == 1:
                    dbg_ap = vp[0][:]

            # ---------- scores + softmax ----------
            _lv = {"qk": 0, "t1": 1, "tr": 2, "B": 3, "sm": 4, "all": 9}[sub]
            if phase >= 2:
                with (
                    tc.tile_pool(name="pscA", bufs=1, space="PSUM") as pscA,
                    tc.tile_pool(name="pscB", bufs=2, space="PSUM") as pscB,
                ):
                    A = pscA.tile([128, H, L], F32, tag="A", name="A")    # 4 banks
                    # mask seeds each bank (start=True covers 2 heads)
                    for h2 in range(0, H, 2):
                        nc.tensor.matmul(A[:, h2:h2 + 2, :], bC[:, C_ONES:C_ONES + LH],
                                         bC[:, C_MASK + h2 * L:C_MASK + (h2 + 2) * L],
                                         start=True, stop=False)
                    for h in range(H):
                        nc.tensor.matmul(A[:, h, :], qf2[:, h, :], kf2[:, h, :],
                                         start=False, stop=False)

                    # term2 first: B[l, m, h], one matmul per key m-pair, so
                    # B_sb is ready when the first A bank closes; two m-halves
                    # keep each psum tile at 2 banks for double-buffering
                    B_sb = work.tile([128, L, H], F32, tag="B_sb", name="B_sb")
                    for half in range(2 if _lv >= 3 else 0):
                        Bp = pscB.tile([128, LH, H], F32, tag="big", name="big")
                        for pp in range(LH // 2):
                            p = half * (LH // 2) + pp
                            nc.tensor.matmul(Bp[:, 2 * pp:2 * pp + 2, :], o2t[:, p, :],
                                             p2c2[:, :, p],
                                             start=(pp % 32 == 0), stop=(pp % 32 == 31))
                        nc.vector.tensor_copy(B_sb[:, half * LH:(half + 1) * LH, :], Bp[:])

                    # t1: psum t1T[m, l, h] per m-chunk -> sb -> PE-transpose into A
                    t1s = [work.tile([128, LH, H], F32, tag=f"t1s{mc}", name=f"t1s{mc}")
                           for mc in range(2)]
                    for mc in range(2 if _lv >= 1 else 0):
                        t1 = pscB.tile([128, LH, H], F32, tag="big", name="big")
                        for p in range(LH // 2):
                            nc.tensor.matmul(t1[:, 2 * p:2 * p + 2, :],
                                             o1t[:, p, mc * 128:(mc + 1) * 128],
                                             c2p2[:, :, p],
                                             start=(p % 32 == 0), stop=(p % 32 == 31))
                        nc.vector.tensor_copy(t1s[mc][:], t1[:])

                    # transpose t1 into A, closing one bank per head-pair, and
                    # run that pair's softmax immediately (pipelines under PE)
                    p16 = work.tile([128, H, L], F16, tag="p16", name="p16")
                    sums = work.tile([128, H], F32, tag="sums", name="sums")
                    recs = work.tile([128, H], F32, tag="recs", name="recs")
                    for hq in range(4):
                        for h in (2 * hq, 2 * hq + 1):
                            for mc in range(2 if _lv >= 2 else 0):
                                nc.tensor.matmul(A[:, h, mc * 128:(mc + 1) * 128],
                                                 t1s[mc][:, :, h], id32[:],
                                                 is_transpose=True, start=False,
                                                 stop=(mc == 1 and h % 2 == 1))
                        for h in ((2 * hq, 2 * hq + 1) if _lv >= 4 else ()):
                            s = smp.tile([128, L], F32, tag="s", name="s")
                            nc.vector.tensor_add(s[:], A[:, h, :], B_sb[:, :, h])
                            nmax = smp.tile([128, 1], F32, tag="nmax", name="nmax")
                            nc.vector.reduce_max(nmax[:], s[:], axis=AX, negate=True)
                            e = smp.tile([128, L], F32, tag="e", name="e")
                            nc.scalar.activation(e[:], s[:], EXP, bias=nmax[:], scale=1.0,
                                                 accum_out=sums[:, h:h + 1])
                            nc.vector.reciprocal(recs[:, h:h + 1], sums[:, h:h + 1])
                            nc.vector.tensor_scalar_mul(p16[:, h, :], e[:], recs[:, h:h + 1])

                if phase == 2:
                    dbg_ap = {0: A[:, 0, :], 1: t1s[0][:, :, 0], 2: A[:, 0, :],
                              3: B_sb[:, :, 0], 4: p16[:, 0, :], 9: B_sb[:, :, 0]}[_lv]
                if phase == 3:
                    dbg_ap = p16[:, 0, :]

            # ---------- ctx + output projection ----------
            if phase >= 4:
                with tc.tile_pool(name="pctx", bufs=1, space="PSUM") as pctx:
                    # p transposed per head via DMA XBAR (sync/scalar HWDGE)
                    pT = [work.tile([128, H, LH], F16, tag=f"pT{c}", name=f"pT{c}")
                          for c in range(2)]
                    for c in range(2):
                        for h in range(H):
                            eng = nc.sync if (h % 2 == 0) else nc.scalar
                            eng.dma_start(out=pT[c][:, h, :],
                                          in_=p16[:, h, c * 128:(c + 1) * 128],
                                          transpose=True)

                    cp = [pctx.tile([128, 2 * LH], F32, tag=f"cp{hp}", name=f"cp{hp}")
                          for hp in range(4)]
                    nrv = NG if phase >= 5 else 0
                    for hp in range(4):
                        for c in range(2):
                            rhs = pT[c][:, 2 * hp:2 * hp + 2, :].rearrange("p hh l -> p l hh")
                            nc.tensor.matmul(cp[hp][:], vp[c][:, hp * 128:(hp + 1) * 128],
                                             rhs, start=(c == 0),
                                             stop=(c == 1 and nrv == 0))
                    for g in range(nrv):
                        rvt = rvp.tile([128, RVG, 2, D], E3, tag="rv", name="rv")
                        nc.sync.dma_start(out=rvt[:], in_=d_rv[g])
                        for j in range(RVG):
                            l = RVG * g + j
                            for hp in range(4):
                                for c in range(2):
                                    nc.tensor.matmul(
                                        cp[hp][:, 2 * l:2 * l + 2],
                                        rvt[:, j, c, hp * 128:(hp + 1) * 128],
                                        pT[c][:, 2 * hp:2 * hp + 2, l:l + 1],
                                        start=False, stop=(c == 1 and l == LH - 1))

                    ctxT = [work.tile([128, LH], F16, tag=f"ctxT{hp}", name=f"ctxT{hp}")
                            for hp in range(4)]
                    for hp in range(4):
                        nc.vector.tensor_copy(
                            ctxT[hp][0:64, :],
                            cp[hp][0:64, :].rearrange("p (l hh) -> p hh l", hh=2)[:, 0, :])
                        nc.vector.tensor_copy(
                            ctxT[hp][64:128, :],
                            cp[hp][64:128, :].rearrange("p (l hh) -> p hh l", hh=2)[:, 1, :])
                    ops = pctx.tile([128, D], F32, tag="ops", name="ops")
                    for hp in range(4):
                        nc.tensor.matmul(ops[:], ctxT[hp][:],
                                         bA[:, A_WO + hp * 512:A_WO + (hp + 1) * 512],
                                         start=(hp == 0), stop=(hp == 3))
                    out_sb = work.tile([128, D], F32, tag="out_sb", name="out_sb")
                    nc.vector.tensor_copy(out_sb[:], ops[:])
                    nc.sync.dma_start(out=d_out[:, :], in_=out_sb[:])

            if phase < 4:
                dbg = work.tile([128, D], F32, tag="dbg", name="dbg")
                nc.vector.memset(dbg[:], 0.0)
                n = min(int(np.prod(dbg_ap.shape[1:])), D)
                nc.vector.tensor_copy(dbg[:dbg_ap.shape[0], 0:n], dbg_ap[:, 0:n])
                nc.sync.dma_start(out=d_out[:, :], in_=dbg[:])

    nc.finalize()
    return nc


_NC_CACHE = None


def _get_nc():
    global _NC_CACHE
    if _NC_CACHE is None:
        import os
        _NC_CACHE = build_nc(int(os.environ.get("KPHASE", "99")),
                             os.environ.get("KSUB", "all"))
    return _NC_CACHE


def _c4(x):
    # [512, N] -> [128, 4*N] with element (p, c*N+f) = x[c*128+p, f]
    n = x.shape[1]
    return x.reshape(4, 128, n).transpose(1, 0, 2).reshape(128, 4 * n)


def host_prep(inputs, c):
    f16, f32 = np.float16, np.float32
    b, lh = c // 2, c % 2
    l0 = lh * LH
    q = np.asarray(inputs["query"][b], f32)
    k = np.asarray(inputs["key"][b], f32)
    v = np.asarray(inputs["value"][b], f32)
    mask = np.asarray(inputs["mask"][b])
    rp = np.asarray(inputs["rel_pos"][b], np.int64)
    rv = np.asarray(inputs["rel_v"][b], f32)

    WqT = np.asarray(inputs["Wq"], f32).T
    WkT = np.asarray(inputs["Wk"], f32).T
    WvT = np.asarray(inputs["Wv"], f32).T
    WoT = np.asarray(inputs["Wo"], f32).T
    qT = q[l0:l0 + LH].T * SCALE                        # [512, LH]
    kT = k.T                                            # [512, L]
    vT = v.T

    blobA = np.concatenate(
        [_c4(WqT), _c4(WkT), _c4(WvT), _c4(WoT), _c4(qT), _c4(kT), _c4(vT)],
        axis=1).astype(f16)
    assert blobA.shape == (128, A_END)

    blobB = np.concatenate(
        [np.asarray(inputs["rel_k"], f32).transpose(2, 0, 1).reshape(DK, 512),
         np.asarray(inputs["rel_q"], f32).transpose(2, 0, 1).reshape(DK, 512) * SCALE],
        axis=1).astype(f16)

    mrow = np.where(mask, np.float32(MASKVAL), np.float32(0.0))
    blobC = np.concatenate(
        [np.asarray(inputs["bq"], f32) * SCALE,
         np.asarray(inputs["bk"], f32),
         np.asarray(inputs["bv"], f32),
         np.ones(LH, f32), np.ones(L, f32), np.ones(L, f32),
         np.tile(mrow, H),
         np.ones(LH, f32)])[None, :].astype(f16)
    assert blobC.shape == (1, C_END)

    d = {"blobA": blobA, "blobB": blobB, "blobC": blobC}
    rp_c = rp[l0:l0 + LH]
    eye = np.eye(R, dtype=f32)
    O1 = eye[:, rp_c]                                   # [R, LH, L]
    O2 = eye[:, rp_c.T]                                 # [R, L, LH]
    d["O1"] = np.ascontiguousarray(
        O1.reshape(R, LH // 2, 2, L).transpose(2, 0, 1, 3)
        .reshape(128, LH // 2, L)).astype(NP_E3)
    d["O2"] = np.ascontiguousarray(
        O2.reshape(R, L // 2, 2, LH).transpose(2, 0, 1, 3)
        .reshape(128, L // 2, LH)).astype(NP_E3)
    # rel_v: [g, m0, j, c, f] where l = RVG*g + j, m = c*128 + m0
    rv_c = rv[l0:l0 + LH]                               # [LH, L, D]
    d["rv"] = np.ascontiguousarray(
        rv_c.reshape(NG, RVG, 2, 128, D).transpose(0, 3, 1, 2, 4)).astype(NP_E3)
    return d


def kernel(**inputs) -> np.ndarray:
    nc = _get_nc()
    in_maps = [host_prep(inputs, c) for c in range(NCORES)]
    res = run_bass_kernel_spmd(nc, in_maps, core_ids=list(range(NCORES)))
    out = np.zeros((B, L, D), np.float32)
    for c in range(NCORES):
        b, lh = c // 2, c % 2
        out[b, lh * LH:(lh + 1) * LH] = res.results[c]["out"]
    out += np.asarray(inputs["bo"], np.float32)[None, None, :]
    return out
